# revision 26
# baseline (speedup 1.0000x reference)
"""AtlasFreeBrainTransformer Trainium2 kernel.

Host contract: kernel(**inputs) takes the FULL unsharded inputs of
reference.setup_inputs() and returns the FULL (B, 2) float32 output.

Sharding: data-parallel over batch B=8 across the 8 NeuronCores (one
batch element per core, weights replicated, no collectives).  The
valid-node mask reduces over batch; it is computed on the host directly
from the integer index tensor C (a node is nonzero iff its 3x3x3 window
contains any nonzero ROI index in any batch), so no cross-core
reduction is needed on device.

The gather + reduce_window ("construct_brain_map" + "extract_nodes")
is algebraically collapsed into one matmul: for each batch,
  nodes[n, :] = sum_{v in win(n)} F_pad[C[v], :] = (S^T F_emb)[n, :]
where S[r, n] = #{v in win(n) : C[v] == r+1} is an integer count matrix
built on the host from C (integer index preprocessing only; all float
math runs on device).

Device pipeline per core (all-transposed layout, x_T is (EMB, T), so
every matmul contraction sits on the partition axis and no on-device
transposes are needed):
  embed FFN -> F_emb (400, 360); nodes_T = S_T-contraction -> x_T
  -> DEPTH transformer layers: qkv_T, per-head flash-style qk/exp/av
     with the softmax denominator accumulated via a ones-column
     appended to V; reciprocals computed across 128 partitions via a
     DRAM-round-trip reshape; post-norm LN over the partition axis via
     ones-matmul stats
  -> mean over tokens -> 3-layer classifier head -> (2,) logits.
"""

import sys

sys.path.insert(0, "/opt/trn_rl_repo")

import math
from contextlib import ExitStack

import numpy as np
import ml_dtypes

import concourse.bass as bass
import concourse.tile as tile
from concourse import bacc, mybir
from concourse.bass_utils import run_bass_kernel_spmd

F32 = mybir.dt.float32
F32R = mybir.dt.float32r
BF16 = mybir.dt.bfloat16
AF = mybir.ActivationFunctionType
ALU = mybir.AluOpType
AX = mybir.AxisListType

# Model dims (hardcoded per problem spec)
B, NROI, DF, G, EMB, NH, HD, FFD, DEPTH = 8, 400, 512, 25, 360, 4, 90, 2048, 2
KS, ST = 3, 2
NBLK = (G - KS) // ST + 1          # 12
NB = NBLK ** 3                     # 1728 nodes
EPS = 1e-5
H450 = 450                         # embed hidden dim
C1, C2, NCLS = 256, 128, 2

QCH = 432    # fp32 psum chunk that fits one 2KB bank
QH = 864     # attention / LN q chunk (2 banks)
PCH = 512    # matmul sub-chunk inside a 2-bank psum tile (bank boundary)


def chunks(total, size):
    out = []
    s = 0
    while s < total:
        out.append((s, min(size, total - s)))
        s += size
    return out


def _r(ap):
    """Matmul operands are declared float32r at the tensor level."""
    return ap


class Builder:
    def __init__(self, nc, tc, ctx, T, Tpad, flags, dbg=False):
        self.nc = nc
        self.tc = tc
        self.ctx = ctx
        self.T = T
        self.Tpad = Tpad
        self.flags = flags
        self.dbg = dbg
        self.dram = {}

    # ---------- dram declarations ----------
    def din(self, name, shape, dtype=F32):
        t = self.nc.dram_tensor(name, list(shape), dtype, kind="ExternalInput")
        self.dram[name] = t.ap()
        return self.dram[name]

    def dout(self, name, shape, dtype=F32):
        t = self.nc.dram_tensor(name, list(shape), dtype, kind="ExternalOutput")
        self.dram[name] = t.ap()
        return self.dram[name]

    def debug_dump(self, name, parts):
        """parts: list of (row_start, sbuf_ap). Dumps to dram out dbg_<name>."""
        if not self.dbg:
            return
        rows = max(s + ap.shape[0] for s, ap in parts)
        cols = parts[0][1].shape[1] if len(parts[0][1].shape) > 1 else 1
        d = self.dout(f"dbg_{name}", (rows, cols))
        for s, ap in parts:
            self.nc.sync.dma_start(out=d[s : s + ap.shape[0], :], in_=ap)

    # ---------- small helpers ----------
    def load_rows(self, pool, dram_ap, row_chunks, cols, dtype=F32, name="w"):
        tiles = []
        for i, (s, sz) in enumerate(row_chunks):
            t = pool.tile([sz, cols], dtype, name=f"{name}{i}", tag=f"{name}{i}")
            self.nc.sync.dma_start(out=t, in_=dram_ap[s : s + sz, :])
            tiles.append(t)
        return tiles

    def load_cols(self, pool, dram_ap, row_chunks, dtype=F32, name="c"):
        return self.load_rows(pool, dram_ap, row_chunks, 1, dtype=dtype,
                              name=name)

    # ---------- build ----------
    def build(self):
        nc, tc, ctx = self.nc, self.tc, self.ctx

        consts = ctx.enter_context(tc.tile_pool(name="consts", bufs=1))
        ones_col = self.load_rows(consts, self.din("ones_col", (128, 1), F32R),
                                  [(0, 128)], 1, dtype=F32R, name="ones_col")[0]
        ones_row = self.load_rows(consts, self.din("ones_row", (1, 128), F32R),
                                  [(0, 1)], 128, dtype=F32R, name="ones_row")[0]
        self._ones_row = ones_row
        self._ones_col = ones_col
        e_sel = self.load_rows(consts, self.din("e_sel", (NH, EMB), F32R), [(0, NH)],
                               EMB, dtype=F32R, name="e_sel")[0]
        self._e_sel = e_sel

        self.dscr = ctx.enter_context(
            tc.tile_pool(name="dscr", bufs=1, space="DRAM"))

        xch = chunks(EMB, 120)
        xpool = ctx.enter_context(tc.tile_pool(name="xpool", bufs=2))

        xt = self.phase_embed_nodes(xpool, xch)

        for l in range(DEPTH):
            xt = self.phase_layer(l, xt, xpool, xch)

        self.phase_head(xt, xch)

    # ---------- phase A/B: embed + nodes ----------
    def phase_embed_nodes(self, xpool, xch):
        nc, tc = self.nc, self.tc
        Tpad = self.Tpad
        f = self.flags
        ones_row = self._ones_row

        w1d = self.din("w1", (DF, H450), F32R)
        b1d = self.din("b1c", (H450, 1))
        w2d = self.din("w2", (H450, EMB), F32R)
        b2d = self.din("b2r", (1, EMB), F32R)
        frd = self.din("f_roiT", (DF, NROI), F32R)
        std = self.din("s_t", (NROI, Tpad), F32R)

        kch_df = chunks(DF, 128)
        mch_450 = chunks(H450, 128)
        mch_400 = chunks(NROI, 128)

        xt = [xpool.tile([msz, Tpad], F32R, name=f"xt{mi}", tag=f"xt{mi}")
              for mi, (ms, msz) in enumerate(xch)]

        with ExitStack() as es:
            epool = es.enter_context(tc.tile_pool(name="embed", bufs=1))
            epsum = es.enter_context(
                tc.tile_pool(name="embed_ps", bufs=2, space="PSUM"))

            w1t = self.load_rows(epool, w1d, kch_df, H450, dtype=F32R, name="w1t")
            frt = self.load_rows(epool, frd, kch_df, NROI, dtype=F32R, name="frt")
            b1c = self.load_cols(epool, b1d, mch_450, name="b1c")

            g = []
            for mi, (ms, msz) in enumerate(mch_450):
                ps = epsum.tile([128, NROI], F32, name=f"psA{mi}", tag="psA")
                for ki in range(len(kch_df)):
                    nc.tensor.matmul(ps[:msz], _r(w1t[ki][:, ms : ms + msz]),
                                     _r(frt[ki]), start=(ki == 0),
                                     stop=(ki == len(kch_df) - 1))
                gt = epool.tile([msz, NROI], F32R, name=f"g{mi}", tag=f"g{mi}")
                nc.scalar.activation(gt, ps[:msz], AF.Gelu, bias=b1c[mi])
                g.append(gt)

            w2t = self.load_rows(epool, w2d, mch_450, EMB, dtype=F32R, name="w2t")
            b2r = self.load_rows(epool, b2d, [(0, 1)], EMB, dtype=F32R, name="b2r")[0]
            femb = []
            for mi, (ms, msz) in enumerate(mch_400):
                ps = epsum.tile([128, EMB], F32, name=f"psB{mi}", tag="psB")
                nk = len(mch_450)
                for ki in range(nk):
                    nc.tensor.matmul(ps[:msz], _r(g[ki][:, ms : ms + msz]),
                                     _r(w2t[ki]), start=(ki == 0),
                                     stop=(ki == nk - 1 and not f["use_b2"]))
                if f["use_b2"]:
                    nc.tensor.matmul(ps[:msz], _r(ones_row[:, :msz]), _r(b2r),
                                     start=False, stop=True)
                ft = epool.tile([msz, EMB], F32R, name=f"femb{mi}",
                                tag=f"femb{mi}")
                nc.vector.tensor_copy(ft, ps[:msz])
                femb.append(ft)

            if self.dbg:
                self.debug_dump("femb",
                                [(s, t) for (s, _), t in zip(mch_400, femb)])

            spool = es.enter_context(tc.tile_pool(name="spool", bufs=2))
            npsum = es.enter_context(
                tc.tile_pool(name="nodes_ps", bufs=2, space="PSUM"))
            for qs, qsz in chunks(Tpad, QCH):
                sts = []
                for ki, (ks, ksz) in enumerate(mch_400):
                    st = spool.tile([ksz, qsz], F32R, name=f"st{ki}",
                                    tag=f"st{ki}")
                    nc.sync.dma_start(out=st,
                                      in_=std[ks : ks + ksz, qs : qs + qsz])
                    sts.append(st)
                for mi, (ms, msz) in enumerate(xch):
                    ps = npsum.tile([128, QCH], F32, name=f"psN{mi}", tag="psN")
                    for ki in range(len(mch_400)):
                        nc.tensor.matmul(ps[:msz, :qsz],
                                         _r(femb[ki][:, ms : ms + msz]),
                                         _r(sts[ki]), start=(ki == 0),
                                         stop=(ki == len(mch_400) - 1))
                    nc.vector.tensor_copy(xt[mi][:, qs : qs + qsz],
                                          ps[:msz, :qsz])

        if self.dbg:
            self.debug_dump("tokens", [(s, t) for (s, _), t in zip(xch, xt)])
        return xt

    # ---------- transformer layer ----------
    def phase_layer(self, l, xt, xpool, xch):
        nc, tc = self.nc, self.tc
        Tpad = self.Tpad
        f = self.flags
        ones_row = self._ones_row

        wqkvd = self.din(f"wqkv{l}", (EMB, 3 * EMB), F32R)
        bqkvd = self.din(f"bqkv{l}c", (3 * EMB, 1))
        bqkvvd = self.din(f"bqkv{l}vr", (1, EMB), F32R)
        wod = self.din(f"wo{l}", (EMB, EMB), F32R)
        bod = self.din(f"bo{l}c", (EMB, 1))
        ln1sd = self.din(f"ln1s{l}c", (EMB, 1))
        ln1bd = self.din(f"ln1b{l}c", (EMB, 1))
        wf1d = self.din(f"wf1_{l}", (EMB, FFD), F32R)
        bf1d = self.din(f"bf1_{l}c", (FFD, 1))
        wf2d = self.din(f"wf2_{l}", (FFD, EMB), F32R)
        bf2d = self.din(f"bf2_{l}c", (EMB, 1))
        ln2sd = self.din(f"ln2s{l}c", (EMB, 1))
        ln2bd = self.din(f"ln2b{l}c", (EMB, 1))

        tch = chunks(Tpad, 128)
        qch = chunks(Tpad, QCH)
        qhch = chunks(Tpad, QH)
        scale = 1.0 / math.sqrt(HD)

        with ExitStack() as es:
            wpool = es.enter_context(tc.tile_pool(name=f"w{l}", bufs=1))
            apool = es.enter_context(tc.tile_pool(name=f"attn{l}", bufs=1))

            wq = self.load_rows(wpool, wqkvd, xch, 3 * EMB, dtype=F32R, name=f"wq{l}")
            # q/k bias in head-aligned (90,1) column tiles
            bqkvc = self.load_cols(wpool, bqkvd, chunks(2 * EMB, HD),
                                   name=f"bqc{l}")
            wo = self.load_rows(wpool, wod, chunks(EMB, HD), EMB, dtype=F32R,
                                name=f"wo{l}")
            boc = self.load_cols(wpool, bod, xch, name=f"boc{l}")
            bqv = self.load_rows(wpool, bqkvvd, [(0, 1)], EMB, dtype=F32R,
                                 name=f"bqv{l}")[0]

            # ---- Q_T / K_T for all heads (90, Tpad) bf16 ----
            qkt = {}
            with ExitStack() as qes:
                qkv_ps = qes.enter_context(
                    tc.tile_pool(name=f"qkvps{l}", bufs=2, space="PSUM"))
                for h in range(NH):
                    for nm, base in (("q", h * HD), ("k", EMB + h * HD)):
                        dst = apool.tile([HD, Tpad], BF16, name=f"{nm}T{h}",
                                         tag=f"{nm}T{h}")
                        for qs, qsz in qch:
                            ps = qkv_ps.tile([HD, QCH], F32, name="psQK",
                                             tag="psQK")
                            for ki in range(len(xch)):
                                nc.tensor.matmul(
                                    ps[:, :qsz],
                                    _r(wq[ki][:, base : base + HD]),
                                    _r(xt[ki][:, qs : qs + qsz]),
                                    start=(ki == 0), stop=(ki == len(xch) - 1))
                            # fold per-row qkv bias into the psum->sbuf copy
                            bcol = self._col_slice(bqkvc, base, HD)
                            nc.vector.tensor_scalar(dst[:, qs : qs + qsz],
                                                    ps[:, :qsz], bcol, None,
                                                    op0=ALU.add)
                        qkt[nm, h] = dst

                # ---- V_ext (t, 4*(HD+1)) bf16, ones col per head ----
                vx = []
                for ti, (ts, tsz) in enumerate(tch):
                    ps = qkv_ps.tile([128, EMB], F32, name=f"psV{ti % 2}",
                                     tag=f"psV{ti % 2}")
                    for ki in range(len(xch)):
                        nc.tensor.matmul(
                            ps[:tsz], _r(xt[ki][:, ts : ts + tsz]),
                            _r(wq[ki][:, 2 * EMB : 3 * EMB]), start=(ki == 0),
                            stop=(ki == len(xch) - 1 and not f["use_bqkv"]))
                    if f["use_bqkv"]:
                        nc.tensor.matmul(ps[:tsz], _r(ones_row[:, :tsz]),
                                         _r(bqv), start=False, stop=True)
                    vt = apool.tile([tsz, NH * (HD + 1)], BF16, name=f"vx{ti}",
                                    tag=f"vx{ti}")
                    vt3 = vt.rearrange("p (h d) -> p h d", h=NH)
                    nc.vector.memset(vt3[:, :, HD : HD + 1], 1.0)
                    nc.vector.tensor_copy(
                        vt3[:, :, :HD],
                        ps[:tsz].rearrange("p (h d) -> p h d", h=NH))
                    vx.append(vt)

            # ---- attention ----
            osb = [apool.tile([HD + 1, Tpad], F32R, name=f"osb{h}",
                              tag=f"osb{h}") for h in range(NH)]

            with ExitStack() as aes:
                att_ps = aes.enter_context(
                    tc.tile_pool(name=f"attps{l}", bufs=1, space="PSUM"))
                exp_pool = aes.enter_context(
                    tc.tile_pool(name=f"exp{l}", bufs=3))
                for hp in range(NH // 2):
                    heads = (2 * hp, 2 * hp + 1)
                    for qhs, qhsz in qhch:
                        pso = {h: att_ps.tile([HD + 1, QH], F32,
                                              name=f"pso{h}",
                                              tag=f"pso{h & 1}")
                               for h in heads}
                        nkt = len(tch)
                        for kti, (kts, ktsz) in enumerate(tch):
                            for h in heads:
                                pss = att_ps.tile([128, QH], F32,
                                                  name=f"pss{h}",
                                                  tag=f"pss{h & 1}")
                                for ss, ssz in chunks(qhsz, PCH):
                                    nc.tensor.matmul(
                                        pss[:ktsz, ss : ss + ssz],
                                        qkt["k", h][:, kts : kts + ktsz],
                                        qkt["q", h][:,
                                                    qhs + ss : qhs + ss + ssz],
                                        start=True, stop=True)
                                et = exp_pool.tile([128, QH], BF16,
                                                   name=f"et{h}",
                                                   tag=f"et{h & 1}")
                                nc.scalar.activation(et[:ktsz, :qhsz],
                                                     pss[:ktsz, :qhsz],
                                                     AF.Exp, scale=scale)
                                for ss, ssz in chunks(qhsz, PCH):
                                    nc.tensor.matmul(
                                        pso[h][:, ss : ss + ssz],
                                        vx[kti][:ktsz, h * (HD + 1) :
                                                (h + 1) * (HD + 1)],
                                        et[:ktsz, ss : ss + ssz],
                                        start=(kti == 0),
                                        stop=(kti == nkt - 1))
                        for h in heads:
                            nc.vector.tensor_copy(
                                osb[h][:, qhs : qhs + qhsz], pso[h][:, :qhsz])

            # ---- 1/denominator across 128 partitions (DRAM reshape) ----
            nw = Tpad // 32
            den_d = self.dscr.tile([NH, 1, Tpad], F32R, name=f"den_d{l}")
            for h in range(NH):
                nc.sync.dma_start(out=den_d[h],
                                  in_=osb[h][HD : HD + 1, :])
            d128 = apool.tile([128, nw], F32R)
            nc.sync.dma_start(
                out=d128,
                in_=den_d.rearrange("h o (p w) -> (h o p) w", p=32))
            r128 = apool.tile([128, nw], F32R)
            nc.vector.reciprocal(r128, d128)
            rec_d = self.dscr.tile([NH, 32, nw], F32R, name=f"rec_d{l}")
            nc.sync.dma_start(out=rec_d.rearrange("h p w -> (h p) w"),
                              in_=r128)
            recT = apool.tile([NH, Tpad], F32R)
            nc.sync.dma_start(out=recT.rearrange("h (p w) -> h p w", p=32),
                              in_=rec_d)

            # ---- normalize O in place by recip (broadcast via matmul) ----
            with ExitStack() as nes:
                rb_ps = nes.enter_context(
                    tc.tile_pool(name=f"rbps{l}", bufs=2, space="PSUM"))
                for h in range(NH):
                    for qs, qsz in qch:
                        ps = rb_ps.tile([HD, QCH], F32, name="psRB",
                                        tag="psRB")
                        nc.tensor.matmul(
                            ps[:, :qsz],
                            _r(self._e_sel[:, h * HD : (h + 1) * HD]),
                            _r(recT[:, qs : qs + qsz]), start=True, stop=True)
                        nc.vector.tensor_tensor(osb[h][:HD, qs : qs + qsz],
                                                osb[h][:HD, qs : qs + qsz],
                                                ps[:, :qsz], op=ALU.mult)

            # ---- output projection + residual -> z ----
            z = [xpool.tile([msz, Tpad], F32R, name=f"z{l}_{mi}",
                            tag=f"xt{mi}") for mi, (ms, msz) in enumerate(xch)]
            with ExitStack() as pes:
                pj_ps = pes.enter_context(
                    tc.tile_pool(name=f"pjps{l}", bufs=3, space="PSUM"))
                for mi, (ms, msz) in enumerate(xch):
                    for qs, qsz in qch:
                        ps = pj_ps.tile([128, QCH], F32, name="psPJ",
                                        tag="psPJ")
                        for h in range(NH):
                            nc.tensor.matmul(ps[:msz, :qsz],
                                             _r(wo[h][:, ms : ms + msz]),
                                             _r(osb[h][:HD, qs : qs + qsz]),
                                             start=(h == 0),
                                             stop=(h == NH - 1))
                        nc.vector.tensor_tensor(z[mi][:, qs : qs + qsz],
                                                ps[:msz, :qsz],
                                                xt[mi][:, qs : qs + qsz],
                                                op=ALU.add)
                        if f["use_bo"]:
                            nc.vector.tensor_scalar(z[mi][:, qs : qs + qsz],
                                                    z[mi][:, qs : qs + qsz],
                                                    boc[mi], None, op0=ALU.add)

        if self.dbg:
            self.debug_dump(f"z{l}", [(s, t) for (s, _), t in zip(xch, z)])

        y = self.emit_ln(f"ln1_{l}", z, xpool, xch, ln1sd, ln1bd,
                         f["ln1_trivial"][l])
        if self.dbg:
            self.debug_dump(f"y{l}", [(s, t) for (s, _), t in zip(xch, y)])

        # ---- FFN ----
        with ExitStack() as es:
            fpool = es.enter_context(tc.tile_pool(name=f"ffn{l}", bufs=1))
            f1_ps = es.enter_context(
                tc.tile_pool(name=f"f1ps{l}", bufs=4, space="PSUM"))
            f2_ps = es.enter_context(
                tc.tile_pool(name=f"f2ps{l}", bufs=1, space="PSUM"))
            hpool = es.enter_context(tc.tile_pool(name=f"hp{l}", bufs=3))

            fch = chunks(FFD, 128)
            wf1 = self.load_rows(fpool, wf1d, xch, FFD, dtype=F32R, name=f"wf1_{l}")
            wf2 = self.load_rows(fpool, wf2d, fch, EMB, dtype=F32R, name=f"wf2_{l}")
            bf1c = self.load_cols(fpool, bf1d, fch, name=f"bf1c{l}")
            bf2c = self.load_cols(fpool, bf2d, xch, name=f"bf2c{l}")

            z2 = [xpool.tile([msz, Tpad], F32R, name=f"z2_{l}_{mi}",
                             tag=f"xt{mi}") for mi, (ms, msz) in enumerate(xch)]
            for qs, qsz in chunks(Tpad, QCH):
                ps2 = [f2_ps.tile([128, QCH], F32, name=f"psF2_{mi}",
                                  tag=f"psF2_{mi}") for mi in range(len(xch))]
                for m, (fs, fsz) in enumerate(fch):
                    ps = f1_ps.tile([128, QCH], F32, name="psF1", tag="psF1")
                    for ki in range(len(xch)):
                        nc.tensor.matmul(ps[:fsz, :qsz],
                                         _r(wf1[ki][:, fs : fs + fsz]),
                                         _r(y[ki][:, qs : qs + qsz]),
                                         start=(ki == 0),
                                         stop=(ki == len(xch) - 1))
                    ht = hpool.tile([128, QCH], F32R, name="ht", tag="ht")
                    nc.scalar.activation(ht[:fsz, :qsz], ps[:fsz, :qsz],
                                         AF.Gelu, bias=bf1c[m])
                    for mi, (ms, msz) in enumerate(xch):
                        nc.tensor.matmul(ps2[mi][:msz, :qsz],
                                         _r(wf2[m][:, ms : ms + msz]),
                                         _r(ht[:fsz, :qsz]), start=(m == 0),
                                         stop=(m == len(fch) - 1))
                for mi, (ms, msz) in enumerate(xch):
                    nc.vector.tensor_tensor(z2[mi][:, qs : qs + qsz],
                                            ps2[mi][:msz, :qsz],
                                            y[mi][:, qs : qs + qsz],
                                            op=ALU.add)
                    if f["use_bf2"]:
                        nc.vector.tensor_scalar(z2[mi][:, qs : qs + qsz],
                                                z2[mi][:, qs : qs + qsz],
                                                bf2c[mi], None, op0=ALU.add)

        xnew = self.emit_ln(f"ln2_{l}", z2, xpool, xch, ln2sd, ln2bd,
                            f["ln2_trivial"][l])
        if self.dbg:
            self.debug_dump(f"x{l + 1}",
                            [(s, t) for (s, _), t in zip(xch, xnew)])
        return xnew

    def _col_slice(self, col_tiles, start, size):
        """Slice (start, size) out of a list of HD-aligned column tiles."""
        i, o = divmod(start, HD)
        assert o == 0 and size == HD, "col slice must be head aligned"
        return col_tiles[i]

    # ---------- layernorm over partition (EMB) axis ----------
    def emit_ln(self, name, z, xpool, xch, sd, bd, trivial):
        nc, tc = self.nc, self.tc
        Tpad = self.Tpad
        nw = Tpad // 32
        inv_d = 1.0 / EMB
        ones_col = self._ones_col
        ones_row = self._ones_row

        y = [xpool.tile([msz, Tpad], F32R, name=f"{name}_y{mi}", tag=f"xt{mi}")
             for mi, (ms, msz) in enumerate(xch)]

        with ExitStack() as es:
            lpool = es.enter_context(tc.tile_pool(name=name, bufs=1))
            sq_pool = es.enter_context(tc.tile_pool(name=f"{name}sq", bufs=2))
            st_ps = es.enter_context(
                tc.tile_pool(name=f"{name}ps", bufs=1, space="PSUM"))

            sum_t = lpool.tile([1, Tpad], F32)
            sq_t = lpool.tile([1, Tpad], F32)
            for qhs, qhsz in chunks(Tpad, QH):
                psm = st_ps.tile([1, QH], F32, name="psm", tag="psm")
                pssq = st_ps.tile([1, QH], F32, name="pssq", tag="pssq")
                for mi, (ms, msz) in enumerate(xch):
                    sq = sq_pool.tile([msz, QH], F32R, name="sq", tag=f"sq{mi}")
                    nc.scalar.activation(sq[:, :qhsz],
                                         z[mi][:, qhs : qhs + qhsz], AF.Square)
                    for ss, ssz in chunks(qhsz, PCH):
                        nc.tensor.matmul(
                            psm[:, ss : ss + ssz], _r(ones_col[:msz, :]),
                            _r(z[mi][:, qhs + ss : qhs + ss + ssz]),
                            start=(mi == 0), stop=(mi == len(xch) - 1))
                        nc.tensor.matmul(
                            pssq[:, ss : ss + ssz], _r(ones_col[:msz, :]),
                            _r(sq[:, ss : ss + ssz]),
                            start=(mi == 0), stop=(mi == len(xch) - 1))
                nc.vector.tensor_copy(sum_t[:, qhs : qhs + qhsz],
                                      psm[:, :qhsz])
                nc.vector.tensor_copy(sq_t[:, qhs : qhs + qhsz],
                                      pssq[:, :qhsz])

            # move (2, Tpad) stat rows onto 32 partitions via DRAM
            st_d = self.dscr.tile([2, 1, Tpad], F32, name=f"{name}_std")
            for i, t in enumerate((sum_t, sq_t)):
                nc.sync.dma_start(out=st_d[i], in_=t)
            st32 = lpool.tile([32, 2 * nw], F32)
            nc.sync.dma_start(
                out=st32.rearrange("p (i w) -> p i w", i=2),
                in_=st_d.rearrange("i o (p w) -> p i (o w)", p=32))

            mean = lpool.tile([32, nw], F32)
            nc.vector.tensor_scalar(mean, st32[:, 0:nw], inv_d, None,
                                    op0=ALU.mult)
            v0 = lpool.tile([32, nw], F32)
            nc.vector.tensor_scalar(v0, st32[:, nw : 2 * nw], inv_d, EPS,
                                    op0=ALU.mult, op1=ALU.add)
            m2 = lpool.tile([32, nw], F32)
            nc.vector.tensor_tensor(m2, mean, mean, op=ALU.mult)
            var = lpool.tile([32, nw], F32)
            nc.vector.tensor_tensor(var, v0, m2, op=ALU.subtract)
            std = lpool.tile([32, nw], F32)
            nc.scalar.activation(std, var, AF.Sqrt)
            ab = lpool.tile([32, 2 * nw], F32R)
            nc.vector.reciprocal(ab[:, 0:nw], std)
            nc.vector.tensor_tensor(ab[:, nw : 2 * nw], mean, ab[:, 0:nw],
                                    op=ALU.mult)
            nc.vector.tensor_scalar(ab[:, nw : 2 * nw], ab[:, nw : 2 * nw],
                                    -1.0, None, op0=ALU.mult)

            ab_d = self.dscr.tile([32, 2, nw], F32R, name=f"{name}_abd")
            nc.sync.dma_start(out=ab_d,
                              in_=ab.rearrange("p (i w) -> p i w", i=2))
            a2 = lpool.tile([1, Tpad], F32R)
            b2 = lpool.tile([1, Tpad], F32R)
            for i, t in enumerate((a2, b2)):
                nc.sync.dma_start(
                    out=t.rearrange("o (p w) -> o p w", p=32),
                    in_=ab_d[:, i : i + 1, :].rearrange("p i w -> i p w"))

            if not trivial:
                sc = self.load_cols(lpool, sd, xch, name=f"{name}s")
                bc = self.load_cols(lpool, bd, xch, name=f"{name}b")
            tmp_pool = es.enter_context(tc.tile_pool(name=f"{name}t", bufs=3))
            with ExitStack() as aes:
                ab_ps = aes.enter_context(
                    tc.tile_pool(name=f"{name}abps", bufs=2, space="PSUM"))
                for qs, qsz in chunks(Tpad, QCH):
                    psa = ab_ps.tile([128, QCH], F32, name="psa", tag="psa")
                    psb = ab_ps.tile([128, QCH], F32, name="psb", tag="psb")
                    nc.tensor.matmul(psa[:120, :qsz], _r(ones_row[:, :120]),
                                     _r(a2[:, qs : qs + qsz]), start=True,
                                     stop=True)
                    nc.tensor.matmul(psb[:120, :qsz], _r(ones_row[:, :120]),
                                     _r(b2[:, qs : qs + qsz]), start=True,
                                     stop=True)
                    for mi, (ms, msz) in enumerate(xch):
                        tmp = tmp_pool.tile([128, QCH], F32, name="lt",
                                            tag="lt")
                        nc.vector.tensor_tensor(tmp[:msz, :qsz],
                                                z[mi][:, qs : qs + qsz],
                                                psa[:msz, :qsz], op=ALU.mult)
                        nc.vector.tensor_tensor(y[mi][:, qs : qs + qsz],
                                                tmp[:msz, :qsz],
                                                psb[:msz, :qsz], op=ALU.add)
                        if not trivial:
                            nc.vector.tensor_scalar(y[mi][:, qs : qs + qsz],
                                                    y[mi][:, qs : qs + qsz],
                                                    sc[mi], bc[mi],
                                                    op0=ALU.mult, op1=ALU.add)
        return y

    # ---------- head ----------
    def phase_head(self, xt, xch):
        nc, tc = self.nc, self.tc
        T = self.T

        cw1d = self.din("cw1", (EMB, C1))
        cb1d = self.din("cb1c", (C1, 1))
        cw2d = self.din("cw2", (C1, C2))
        cb2d = self.din("cb2c", (C2, 1))
        cw3d = self.din("cw3", (C2, NCLS))
        cb3d = self.din("cb3c", (NCLS, 1))
        outd = self.dout("out", (NCLS, 1))

        with ExitStack() as es:
            hpool = es.enter_context(tc.tile_pool(name="head", bufs=1))
            hps = es.enter_context(
                tc.tile_pool(name="head_ps", bufs=2, space="PSUM"))

            hmean = []
            for mi, (ms, msz) in enumerate(xch):
                hm = hpool.tile([msz, 1], F32, name=f"hm{mi}", tag=f"hm{mi}")
                nc.vector.reduce_sum(hm, xt[mi][:, :T], axis=AX.X)
                nc.vector.tensor_scalar(hm, hm, 1.0 / T, None, op0=ALU.mult)
                hmean.append(hm)
            if self.dbg:
                self.debug_dump("hmean",
                                [(s, t) for (s, _), t in zip(xch, hmean)])

            cw1 = self.load_rows(hpool, cw1d, xch, C1, name="cw1")
            cb1 = self.load_cols(hpool, cb1d, chunks(C1, 128), name="cb1")
            h1 = []
            for mi, (ms, msz) in enumerate(chunks(C1, 128)):
                ps = hps.tile([128, 1], F32, name=f"psH1_{mi}", tag="psH")
                for ki in range(len(xch)):
                    nc.tensor.matmul(ps[:msz], _r(cw1[ki][:, ms : ms + msz]),
                                     _r(hmean[ki]), start=(ki == 0),
                                     stop=(ki == len(xch) - 1))
                ht = hpool.tile([msz, 1], F32, name=f"h1_{mi}", tag=f"h1_{mi}")
                nc.scalar.activation(ht, ps[:msz], AF.Gelu, bias=cb1[mi])
                h1.append(ht)

            cw2 = self.load_rows(hpool, cw2d, chunks(C1, 128), C2, name="cw2")
            cb2 = self.load_cols(hpool, cb2d, [(0, C2)], name="cb2")
            ps = hps.tile([128, 1], F32, name="psH2", tag="psH")
            for ki in range(len(cw2)):
                nc.tensor.matmul(ps[:C2], _r(cw2[ki]), _r(h1[ki]),
                                 start=(ki == 0), stop=(ki == len(cw2) - 1))
            h2 = hpool.tile([C2, 1], F32)
            nc.scalar.activation(h2, ps[:C2], AF.Relu, bias=cb2[0])

            cw3 = self.load_rows(hpool, cw3d, [(0, C2)], NCLS, name="cw3")
            cb3 = self.load_cols(hpool, cb3d, [(0, NCLS)], name="cb3")
            ps3 = hps.tile([128, 1], F32, name="psH3", tag="psH")
            nc.tensor.matmul(ps3[:NCLS], _r(cw3[0]), _r(h2), start=True,
                             stop=True)
            res = hpool.tile([NCLS, 1], F32)
            nc.scalar.activation(res, ps3[:NCLS], AF.Identity, bias=cb3[0])
            nc.sync.dma_start(out=outd, in_=res)


# ---------------------------------------------------------------------------
# Host side
# ---------------------------------------------------------------------------

def _build_counts(C):
    """S[b, r, n] = #{v in win(n): C[b, v] == r} for r in 0..NROI."""
    Bn = C.shape[0]
    S = np.zeros((Bn, NROI + 1, NB), np.int32)
    b_idx = np.arange(Bn)[:, None]
    n_idx = np.arange(NB)[None, :]
    for di in range(KS):
        for dj in range(KS):
            for dk in range(KS):
                sub = C[:, di : di + 2 * (NBLK - 1) + 1 : ST,
                        dj : dj + 2 * (NBLK - 1) + 1 : ST,
                        dk : dk + 2 * (NBLK - 1) + 1 : ST].reshape(Bn, NB)
                np.add.at(S, (b_idx, sub, n_idx), 1)
    return S


def host_prepare(inputs):
    inp = {k: np.asarray(v) for k, v in inputs.items()}
    F_roi = inp["F_roi"].astype(np.float32)
    C = inp["C"].astype(np.int64)

    S = _build_counts(C)
    valid = S[:, 1:, :].sum(axis=(0, 1)) > 0
    vidx = np.nonzero(valid)[0]
    T = int(len(vidx))
    Tpad = ((T + 31) // 32) * 32
    s_t = np.zeros((C.shape[0], NROI, Tpad), np.float32)
    s_t[:, :, :T] = S[:, 1:, :][:, :, vidx].astype(np.float32)

    f32 = lambda x: np.ascontiguousarray(np.asarray(x), dtype=np.float32)
    col = lambda x: f32(x).reshape(-1, 1)
    row = lambda x: f32(x).reshape(1, -1)

    e_sel = np.zeros((NH, EMB), np.float32)
    for h in range(NH):
        e_sel[h, h * HD : (h + 1) * HD] = 1.0

    shared = {
        "w1": f32(inp["ffn_w1"]), "b1c": col(inp["ffn_b1"]),
        "w2": f32(inp["ffn_w2"]), "b2r": row(inp["ffn_b2"]),
        "cw1": f32(inp["cw1"]), "cb1c": col(inp["cb1"]),
        "cw2": f32(inp["cw2"]), "cb2c": col(inp["cb2"]),
        "cw3": f32(inp["cw3"]), "cb3c": col(inp["cb3"]),
        "e_sel": e_sel,
        "ones_col": np.ones((128, 1), np.float32),
        "ones_row": np.ones((1, 128), np.float32),
    }
    for l in range(DEPTH):
        shared[f"wqkv{l}"] = f32(inp["wqkv"][l])
        shared[f"bqkv{l}c"] = col(inp["bqkv"][l])
        shared[f"bqkv{l}vr"] = row(inp["bqkv"][l][2 * EMB :])
        shared[f"wo{l}"] = f32(inp["wo"][l])
        shared[f"bo{l}c"] = col(inp["bo"][l])
        shared[f"ln1s{l}c"] = col(inp["ln1_s"][l])
        shared[f"ln1b{l}c"] = col(inp["ln1_b"][l])
        shared[f"wf1_{l}"] = f32(inp["wf1"][l])
        shared[f"bf1_{l}c"] = col(inp["bf1"][l])
        shared[f"wf2_{l}"] = f32(inp["wf2"][l])
        shared[f"bf2_{l}c"] = col(inp["bf2"][l])
        shared[f"ln2s{l}c"] = col(inp["ln2_s"][l])
        shared[f"ln2b{l}c"] = col(inp["ln2_b"][l])

    flags = {
        "use_b2": bool(np.any(np.asarray(inp["ffn_b2"]) != 0)),
        "use_bqkv": bool(np.any(np.asarray(inp["bqkv"]) != 0)),
        "use_bo": bool(np.any(np.asarray(inp["bo"]) != 0)),
        "use_bf2": bool(np.any(np.asarray(inp["bf2"]) != 0)),
        "ln1_trivial": [bool(np.all(np.asarray(inp["ln1_s"][l]) == 1)
                             and np.all(np.asarray(inp["ln1_b"][l]) == 0))
                        for l in range(DEPTH)],
        "ln2_trivial": [bool(np.all(np.asarray(inp["ln2_s"][l]) == 1)
                             and np.all(np.asarray(inp["ln2_b"][l]) == 0))
                        for l in range(DEPTH)],
    }

    in_maps = []
    for b in range(F_roi.shape[0]):
        m = dict(shared)
        m["f_roiT"] = np.ascontiguousarray(F_roi[b].T)
        m["s_t"] = np.ascontiguousarray(s_t[b])
        in_maps.append(m)
    return in_maps, T, Tpad, flags


def build_program(T, Tpad, flags, dbg=False):
    nc = bacc.Bacc("TRN2", target_bir_lowering=False, debug=False,
                   enable_asserts=False, num_devices=B)
    with tile.TileContext(nc) as tc:
        # float32r tiles are 4-byte floats (PE-rounded); the low-precision
        # guard only knows f32. All psum accumulation stays fp32.
        with nc.allow_low_precision("float32r matmul operand plumbing"):
            with ExitStack() as ctx:
                bld = Builder(nc, tc, ctx, T, Tpad, flags, dbg=dbg)
                bld.build()
    nc.compile()
    return nc


def kernel(**inputs):
    in_maps, T, Tpad, flags = host_prepare(inputs)
    nc = build_program(T, Tpad, flags)
    res = run_bass_kernel_spmd(nc, in_maps, core_ids=list(range(len(in_maps))))
    out = np.stack([r["out"].reshape(NCLS) for r in res.results])
    return out.astype(np.float32)
